# revision 12
# baseline (speedup 1.0000x reference)
"""Trainium2 Bass kernel for the neural-renderer loss model (v4).

Pixels are sharded 16 image rows per core across 8 cores.  Each core's
2048 pixels are processed as 16 blocks of 128 pixels (2 rows x 64 cols),
in 2 windows of 8 blocks.

v4 replaces the per-pixel dma_gather (descriptor-overhead-bound, ~8.7us
per window on GpSimd) with exact one-hot matmuls on the Tensor engine:

  1. Raster: per-(block,slot) affine coefficients over the constant pixel
     basis (1, xs0[j], r) -> 4 concurrent fp32r matmuls -> PSUM grids ->
     grouped negated max-reduces -> nk keys -> per-block nkmax.
  2. Winner one-hot WITHOUT max_index: ohp[p, (t,s)] = is_eq(nk, nkmax)
     (GpSimd, off the busy DVE).  Ties/empty blocks resolve to all-zero
     poison rows, so multi-matches are harmless.
  3. PE transposes ohp (2 blocks per [128,128] slice, fp32 via identity)
     -> slot-major one-hot; ScalarE copies PSUM->SBUF in both fp32r
     (for exact coef selection) and bf16 (for cube selection).
  4. Per block-pair: one bf16 matmul selects the 192-value texture cube
     (two concurrent PE row groups, even block on partitions 0-63, odd
     on 64-127) and one tiny fp32r matmul selects the 9 barycentric
     coefs exactly (one-hot weights make the products exact).
  5. ScalarE copies cube PSUM->SBUF bf16; DVE tail: winner barycentrics,
     clip/renorm, fused tent weights min(|3q-k|,1), separable w64,
     bf16 cube multiply + grouped reduce, hit mask, diff vs reference.
  6. Loss: fused diff^2-accumulate on DVE + GpSimd partition_all_reduce;
     host sums the 8 per-core scalars.

The slot tables (cube bf16 + coef fp32) are bulk-DMA'd at t=0 with no
dependencies; rcb/pb DMA only the 12 real partitions (4 row-group
replicas of 3 basis rows) instead of a padded [99,*] tensor.
"""
import numpy as np
import ml_dtypes

H = W = 128
TS = 4
F = 2560
DIST, ELEV, AZIM = 2.732, 0.0, 90.0
NCORES = 8
TPC = H // NCORES            # image rows per core
KSCALE = 1e20
DSHIFT = 4.0                 # small shift keeps depth positive yet precise
HIT_THRESH = 1e6

BR, BC = 2, 64               # block shape (rows x cols), 128 px/block
NBLK = TPC * 128 // (BR * BC)   # 16 blocks per core
CAP = 64                     # face slots per block
NWIN = 2                     # windows (8 blocks each)
WB = NBLK // NWIN            # blocks per window
NPAIR = NBLK // 2            # block pairs (even on part 0-63, odd 64-127)
CUBE = 192                   # cube row (c-major 3*4*4*4)
CROW = 12                    # coef row (9 used + pad)

_prog_cache = {}


def _geom(vertices, faces):
    v64 = np.asarray(vertices[0], np.float64)
    el, az = np.deg2rad(ELEV), np.deg2rad(AZIM)
    eye = DIST * np.array(
        [np.cos(el) * np.sin(az), np.sin(el), -np.cos(el) * np.cos(az)]
    )
    up = np.array([0.0, 1.0, 0.0])
    z = -eye / np.linalg.norm(eye)
    x = np.cross(up, z); x = x / np.linalg.norm(x)
    y = np.cross(z, x)
    R = np.stack([x, y, z])
    vc = (v64 - eye) @ R.T
    tri = vc[np.asarray(faces[0])]               # [F,3,3]
    a, b, c = tri[:, 0], tri[:, 1], tri[:, 2]
    area = (b[:, 0] - a[:, 0]) * (c[:, 1] - a[:, 1]) - \
           (b[:, 1] - a[:, 1]) * (c[:, 0] - a[:, 0])
    sa = np.where(np.abs(area) < 1e-8, 1e-8, area)
    valid = np.abs(area) >= 1e-8

    def edge_coeffs(p, q):
        # edge(p,q,pt) = (qx-px)(pty-py) - (qy-py)(ptx-px) = A + B*ptx + C*pty
        A = p[:, 0] * q[:, 1] - p[:, 1] * q[:, 0]
        B = -(q[:, 1] - p[:, 1])
        C = q[:, 0] - p[:, 0]
        return np.stack([A, B, C])               # [3,F]

    w0c = edge_coeffs(b, c) / sa
    w1c = edge_coeffs(c, a) / sa
    w2c = edge_coeffs(a, b) / sa
    z3 = tri[:, :, 2]
    Dc = w0c * z3[:, 0] + w1c * z3[:, 1] + w2c * z3[:, 2]
    p2x = np.stack([a[:, 0], b[:, 0], c[:, 0]])
    p2y = np.stack([a[:, 1], b[:, 1], c[:, 1]])
    return dict(w0c=w0c, w1c=w1c, w2c=w2c, Dc=Dc, valid=valid,
                bbx=(p2x.min(0), p2x.max(0)), bby=(p2y.min(0), p2y.max(0)))


def _bin_faces(geom):
    """Per-(core, block) conservative face lists. None on CAP overflow."""
    xs = ((np.arange(W, dtype=np.float64) + 0.5) / W * 2.0 - 1.0)
    ys = (1.0 - (np.arange(H, dtype=np.float64) + 0.5) / H * 2.0)
    wcs = [geom["w0c"], geom["w1c"], geom["w2c"]]
    valid = geom["valid"]
    nbr, nbc = H // BR, W // BC
    lists = np.full((NCORES, NBLK, CAP), F, np.int64)   # pad = poison face F
    for bi in range(nbr):
        rcy = ys[bi * BR:(bi + 1) * BR]
        cy = (rcy[0] + rcy[-1]) / 2; hy = abs(rcy[-1] - rcy[0]) / 2
        for bj in range(nbc):
            rcx = xs[bj * BC:(bj + 1) * BC]
            cx = (rcx[0] + rcx[-1]) / 2; hx = (rcx[-1] - rcx[0]) / 2
            ok = valid.copy()
            bbx, bby = geom["bbx"], geom["bby"]
            ok &= (bbx[0] <= cx + hx + 1e-6) & (bbx[1] >= cx - hx - 1e-6)
            ok &= (bby[0] <= cy + hy + 1e-6) & (bby[1] >= cy - hy - 1e-6)
            for e in range(3):
                A, B, C = wcs[e][0], wcs[e][1], wcs[e][2]
                wmax = A + B * cx + C * cy + np.abs(B) * hx + np.abs(C) * hy
                eps = 1e-5 * (np.abs(A) + np.abs(B) + np.abs(C))
                ok &= (wmax + eps) >= 0
            idx = np.nonzero(ok)[0]
            if idx.size > CAP:
                # refine with the exact pixel-center test (+ fp slack)
                px = xs[bj * BC:(bj + 1) * BC]
                py = ys[bi * BR:(bi + 1) * BR]
                PY, PX = np.meshgrid(py, px, indexing="ij")
                P0, P1 = PX.ravel()[None, :], PY.ravel()[None, :]
                ins = np.ones((idx.size, BR * BC), bool)
                for e in range(3):
                    A = wcs[e][0][idx]; B = wcs[e][1][idx]; C = wcs[e][2][idx]
                    eps = 1e-5 * (np.abs(A) + np.abs(B) + np.abs(C))
                    w = A[:, None] + B[:, None] * P0 + C[:, None] * P1
                    ins &= (w + eps[:, None]) >= 0
                idx = idx[ins.any(1)]
                if idx.size > CAP:
                    return None
            core = (bi * BR) // TPC
            blkrow = bi - core * (TPC // BR)
            t = blkrow * nbc + bj
            lists[core, t, :idx.size] = idx
    return lists


def _build_binned(loop_n=None, probes=False, ablate=None):
    """Binned program v4. loop_n wraps the body in a hardware loop."""
    from contextlib import ExitStack
    import concourse.bacc as bacc
    import concourse.tile as tile
    from concourse import mybir
    from concourse._compat import axon_active

    fp32 = mybir.dt.float32
    fp32r = mybir.dt.float32r
    bf16 = mybir.dt.bfloat16
    AL = mybir.AluOpType
    nc = bacc.Bacc(
        "TRN2",
        target_bir_lowering=False,
        debug=not axon_active(),
        num_devices=NCORES,
    )

    GCOLS = CAP * 4                       # grid cols per block
    rcb_in = nc.dram_tensor("rcb", [12, NBLK * GCOLS], fp32r,
                            kind="ExternalInput").ap()
    pb_in = nc.dram_tensor("pb", [12, 128], fp32r,
                           kind="ExternalInput").ap()
    SROW = CUBE + 24          # cube | coef_hi(12) | coef_lo(12), bf16
    tabc_in = nc.dram_tensor("tabc", [128, NBLK * SROW], bf16,
                             kind="ExternalInput").ap()
    # packed small constants: [xs0 | rvec | kk(192) | refsl(48) | ident(128)]
    NCONST = 2 + NBLK * 12 + NBLK * 3 + 128
    cpk_in = nc.dram_tensor("cpk", [128, NCONST], fp32,
                            kind="ExternalInput").ap()
    lossp = nc.dram_tensor("lossp", [1, 1], fp32, kind="ExternalOutput").ap()
    if probes:
        p_nkmax = nc.dram_tensor("p_nkmax", [128, NBLK], fp32,
                                 kind="ExternalOutput").ap()
        p_nk = nc.dram_tensor("p_nk", [128, NBLK * CAP], fp32,
                              kind="ExternalOutput").ap()
        p_oh = nc.dram_tensor("p_oh", [128, NWIN * WB * CAP], fp32,
                              kind="ExternalOutput").ap()
        p_cb = nc.dram_tensor("p_cb", [128, NBLK * CUBE], fp32,
                              kind="ExternalOutput").ap()
        p_diff = nc.dram_tensor("p_diff", [128, NBLK * 3], fp32,
                                kind="ExternalOutput").ap()
        p_acc = nc.dram_tensor("p_acc", [128, 1], fp32,
                               kind="ExternalOutput").ap()

    with tile.TileContext(nc) as tc, ExitStack() as ctx:
        const = ctx.enter_context(tc.tile_pool(name="const", bufs=1))
        sb = ctx.enter_context(tc.tile_pool(name="sb", bufs=3))
        sm = ctx.enter_context(tc.tile_pool(name="sm", bufs=3))
        psg = ctx.enter_context(tc.tile_pool(name="psg", bufs=2,
                                             space="PSUM"))
        pst = ctx.enter_context(tc.tile_pool(name="pst", bufs=2,
                                             space="PSUM"))
        psc = ctx.enter_context(tc.tile_pool(name="psc", bufs=1,
                                             space="PSUM"))

        if loop_n is not None:
            ctx.enter_context(tc.For_i(0, loop_n, 1))

        # ---- inputs: only real partitions; raster operands first ----
        pb_t = const.tile([99, 128], fp32r, tag="pb")
        rcb_t = const.tile([99, NBLK * GCOLS], fp32r, tag="rcb")
        for g in range(4):
            eng = (nc.sync, nc.scalar)[g % 2]
            eng.dma_start(out=pb_t[32 * g:32 * g + 3, :],
                          in_=pb_in[3 * g:3 * g + 3, :])
            eng.dma_start(out=rcb_t[32 * g:32 * g + 3, :],
                          in_=rcb_in[3 * g:3 * g + 3, :])
        cpk = const.tile([128, NCONST], fp32, tag="cpk")
        nc.scalar.dma_start(out=cpk[:], in_=cpk_in[:])
        xs0 = cpk[:, 0:1]
        rvec = cpk[:, 1:2]
        kk = cpk[:, 2:2 + NBLK * 12].rearrange("p (t k) -> p t k", k=4)
        rs = cpk[:, 2 + NBLK * 12:2 + NBLK * 15].rearrange(
            "p (t c) -> p t c", c=3)
        ident = cpk[:, 2 + NBLK * 15:]
        tabc = const.tile([128, NBLK * SROW], bf16, tag="tabc")
        hc = NBLK * SROW // 2
        nc.sync.dma_start(out=tabc[:, 0:hc], in_=tabc_in[:, 0:hc])
        nc.scalar.dma_start(out=tabc[:, hc:], in_=tabc_in[:, hc:])

        # ---- persistent result tiles ----
        nk = const.tile([128, NBLK * CAP], fp32, tag="nk")
        nkmax = const.tile([128, NBLK], fp32, tag="nkmax")
        diff = const.tile([128, NBLK, 3], fp32, tag="diff")

        # window-1 sparse slots stay poisoned (host sorts dense-first)
        nc.gpsimd.memset(nk[:, WB * CAP:], -1e30)

        # ---- raster both windows (PE stays busy while DVE reduces) ----
        ohps = []
        for w in range(NWIN):
            ws = slice(w * WB, (w + 1) * WB)
            wsl = slice(w * WB * CAP, (w + 1) * WB * CAP)
            for q in range(2):
                pw = psg.tile([128, WB * GCOLS // 2], fp32, tag="grid")
                for i in range(2):
                    g = 2 * q + i
                    t = w * WB + 2 * g
                    nc.tensor.matmul(
                        pw[:, i * 512:(i + 1) * 512],
                        lhsT=pb_t[32 * g:32 * g + 3, :],
                        rhs=rcb_t[32 * g:32 * g + 3,
                                  t * GCOLS:(t + 2) * GCOLS],
                        start=True, stop=True,
                        tile_position=(32 * g, 0),
                    )
                hsl = slice((2 * w + q) * 4 * CAP, (2 * w + q + 1) * 4 * CAP)
                if w == 0 and q == 0:
                    # split the first reduce so DVE starts after quad 0
                    for u in range(2):
                        nc.vector.tensor_reduce(
                            nk[:, u * 2 * CAP:(u + 1) * 2 * CAP],
                            pw[:, u * 512:(u + 1) * 512].rearrange(
                                "p (f v) -> p f v", v=4),
                            axis=mybir.AxisListType.X, op=AL.max,
                            negate=True)
                elif w == 0:
                    nc.vector.tensor_reduce(
                        nk[:, hsl],
                        pw[:].rearrange("p (f v) -> p f v", v=4),
                        axis=mybir.AxisListType.X, op=AL.max, negate=True)
                else:
                    # sparse window: real faces sit in slots 0..39
                    nc.vector.tensor_reduce(
                        nk[:, hsl].rearrange("p (t s) -> p t s",
                                             s=CAP)[:, :, 0:40],
                        pw[:].rearrange("p (t c) -> p t c",
                                        c=GCOLS)[:, :, 0:160]
                            .rearrange("p t (f v) -> p t f v", v=4),
                        axis=mybir.AxisListType.X, op=AL.max, negate=True)
            nc.vector.tensor_reduce(
                nkmax[:, ws],
                nk[:, wsl].rearrange("p (t s) -> p t s", s=CAP),
                axis=mybir.AxisListType.X, op=AL.max)
            # winner one-hot, pixel-major (GpSimd: exact fp32 compare)
            ohp = sm.tile([128, WB * CAP], fp32, tag="ohp")
            nc.vector.tensor_tensor(
                ohp[:].rearrange("p (t s) -> p t s", s=CAP),
                nk[:, wsl].rearrange("p (t s) -> p t s", s=CAP),
                nkmax[:, ws].unsqueeze(2).broadcast_to((128, WB, CAP)),
                op=AL.is_equal)
            ohps.append(ohp)
            if probes:
                nc.sync.dma_start(out=p_oh[:, wsl], in_=ohp[:])

        from concourse import bass_isa as _bisa
        if ablate is not None:
            acca = sm.tile([128, 1], fp32, tag="acca")
            nc.vector.scalar_tensor_tensor(
                diff[:, :, 0], nkmax[:], 1.0, nkmax[:], op0=AL.mult,
                op1=AL.mult, accum_out=acca[:])
            lsba = sm.tile([128, 1], fp32, tag="lsba")
            nc.gpsimd.partition_all_reduce(lsba[:], acca[:], channels=128,
                                           reduce_op=_bisa.ReduceOp.add)
            nc.sync.dma_start(out=lossp[:], in_=lsba[0:1, :])

        # ---- one-hot transpose + select + tail, per window ----
        cubeb = const.tile([128, NBLK, CUBE], bf16, tag="cubeb")
        for w in range(NWIN if ablate != "raster" else 0):
            ws = slice(w * WB, (w + 1) * WB)
            ohp = ohps[w]

            psT = pst.tile([128, (WB // 2) * 128], fp32, tag="psT")
            for j in range(WB // 2):
                nc.tensor.transpose(
                    psT[:, j * 128:(j + 1) * 128],
                    ohp[:, j * 128:(j + 1) * 128],
                    ident)
            ohb = sm.tile([128, (WB // 2) * 128], bf16, tag="ohb")
            nc.scalar.activation(ohb[:], psT[:],
                                 mybir.ActivationFunctionType.Copy)

            if ablate == "oh":
                continue
            ckhl = sm.tile([128, WB, 24], fp32, tag="ckhl")
            for h in range(2):
                pc = psc.tile([128, 4, 256], fp32, tag="pc")
                for tt in range(4):
                    tl = 4 * h + tt         # block within window
                    tg = w * WB + tl        # global block
                    j = tl // 2             # pair within window
                    nc.tensor.matmul(
                        pc[:, tt, 0:SROW],
                        lhsT=ohb[:, j * 128:(j + 1) * 128],
                        rhs=tabc[:, tg * SROW:(tg + 1) * SROW],
                        start=True, stop=True)
                nc.scalar.activation(
                    cubeb[:, w * WB + 4 * h:w * WB + 4 * (h + 1), :],
                    pc[:, :, 0:CUBE],
                    mybir.ActivationFunctionType.Copy)
                nc.scalar.activation(
                    ckhl[:, 4 * h:4 * (h + 1), :],
                    pc[:, :, CUBE:SROW],
                    mybir.ActivationFunctionType.Copy)

            if ablate in ("sel", "sel_cube", "sel_coef"):
                continue
            # ---- winner barycentric u_i = clip01(A' + B*xs0 + C'*r) ----
            ck = sm.tile([128, WB, CROW], fp32, tag="ck")
            nc.vector.tensor_tensor(ck[:], ckhl[:, :, 0:CROW],
                                    ckhl[:, :, CROW:24], op=AL.add)
            Av = ck[:, :, 0:9:3]
            Bv = ck[:, :, 1:9:3]
            Cv = ck[:, :, 2:9:3]
            u3 = sm.tile([128, WB, 3], fp32, tag="u3")
            nc.vector.scalar_tensor_tensor(
                u3[:], Bv, xs0, Av, op0=AL.mult, op1=AL.add)
            nc.vector.scalar_tensor_tensor(
                u3[:], Cv, rvec, u3[:], op0=AL.mult, op1=AL.add)
            nc.vector.tensor_scalar(u3[:], u3[:], 0.0, 1.0, AL.max, AL.min)
            ssum = sm.tile([128, WB], fp32, tag="ssum")
            nc.vector.tensor_reduce(ssum[:], u3[:],
                                    axis=mybir.AxisListType.X, op=AL.add)
            nc.vector.tensor_scalar(ssum[:], ssum[:], 1e-8, None, AL.add)
            rcp = sm.tile([128, WB], fp32, tag="rcp")
            nc.vector.reciprocal(rcp[:], ssum[:])
            q3 = sm.tile([128, WB, 3], fp32, tag="q3")
            nc.vector.tensor_tensor(
                q3[:], u3[:],
                rcp[:].unsqueeze(2).broadcast_to((128, WB, 3)), op=AL.mult)

            # ---- tents te = 1 - min(|3q - k|, 1)  (3 fused DVE ops) ----
            d4 = sb.tile([128, WB, 3, 4], fp32, tag="d4")
            te = sb.tile([128, WB, 3, 4], bf16, tag="te")
            nc.vector.scalar_tensor_tensor(
                d4[:].rearrange("p t c k -> p (t c) k"),
                q3[:].rearrange("p t c -> p (t c)").unsqueeze(2)
                    .broadcast_to((128, WB * 3, 4)),
                3.0, kk[:, 3 * w * WB:3 * (w + 1) * WB, :],
                op0=AL.mult, op1=AL.subtract)
            tw = sb.tile([128, WB, 3, 4], fp32, tag="tw")
            nc.vector.tensor_scalar(tw[:], d4[:], -1.0, 1.0, AL.mult, AL.add)
            nc.vector.tensor_scalar(d4[:], d4[:], 1.0, None, AL.add)
            nc.vector.tensor_tensor(d4[:], d4[:], tw[:], op=AL.min)
            nc.vector.tensor_scalar(te[:], d4[:], 0.0, None, AL.max)

            # ---- separable weights w64 = t0 x t1 x t2 ----
            w01 = sb.tile([128, WB, 4, 4], bf16, tag="w01")
            nc.vector.tensor_tensor(
                w01[:],
                te[:, :, 0, :].unsqueeze(3).broadcast_to((128, WB, 4, 4)),
                te[:, :, 1, :].unsqueeze(2).broadcast_to((128, WB, 4, 4)),
                op=AL.mult)
            w64 = sb.tile([128, WB, 16, 4], bf16, tag="w64")
            nc.vector.tensor_tensor(
                w64[:],
                w01[:].rearrange("p t a b -> p t (a b)").unsqueeze(3)
                    .broadcast_to((128, WB, 16, 4)),
                te[:, :, 2, :].unsqueeze(2).broadcast_to((128, WB, 16, 4)),
                op=AL.mult)

            # ---- cube contraction: bf16 multiply + grouped reduce ----
            mb = sb.tile([128, WB, 3, 64], bf16, tag="mb")
            nc.vector.tensor_tensor(
                mb[:],
                cubeb[:, ws, :].rearrange("p t (c s) -> p t c s", s=64),
                w64[:].rearrange("p t a b -> p t (a b)").unsqueeze(2)
                    .broadcast_to((128, WB, 3, 64)),
                op=AL.mult)
            col = sm.tile([128, WB, 3], fp32, tag="col")
            nc.vector.tensor_reduce(
                col[:], mb[:].rearrange("p t c s -> p (t c) s"),
                axis=mybir.AxisListType.X, op=AL.add)

            # ---- hit mask + diff vs reference ----
            hm = sm.tile([128, WB], fp32, tag="hm")
            nc.vector.tensor_scalar(hm[:], nkmax[:, ws], -HIT_THRESH, None,
                                    AL.is_gt)
            flat = sm.tile([128, WB, 3], fp32, tag="flat")
            nc.vector.tensor_tensor(
                flat[:], col[:],
                hm[:].unsqueeze(2).broadcast_to((128, WB, 3)), op=AL.mult)
            nc.vector.tensor_tensor(diff[:, ws, :], flat[:], rs[:, ws, :],
                                    op=AL.subtract)

        # ---- squared-error accumulate (DVE) + partition reduce (Pool) ----
        if ablate is not None:
            sq = None
        else:
            sq = sb.tile([128, NBLK * 3], fp32, tag="sq")
        if sq is not None:
            acc = sm.tile([128, 1], fp32, tag="acc")
            dv = diff[:].rearrange("p t c -> p (t c)")
            nc.vector.scalar_tensor_tensor(sq[:], dv, 1.0, dv, op0=AL.mult,
                                           op1=AL.mult, accum_out=acc[:])
            from concourse import bass_isa
            lsb = sm.tile([128, 1], fp32, tag="lsb")
            nc.gpsimd.partition_all_reduce(lsb[:], acc[:], channels=128,
                                           reduce_op=bass_isa.ReduceOp.add)
            nc.sync.dma_start(out=lossp[:], in_=lsb[0:1, :])
        if probes:
            nc.sync.dma_start(out=p_nkmax[:], in_=nkmax[:])
            nc.sync.dma_start(out=p_nk[:], in_=nk[:])
            nc.sync.dma_start(out=p_cb[:],
                              in_=cubeb[:].rearrange("p t c -> p (t c)"))
            nc.sync.dma_start(out=p_diff[:],
                              in_=diff[:].rearrange("p t c -> p (t c)"))
            nc.sync.dma_start(out=p_acc[:], in_=acc[:])

    nc.compile()
    return nc


def _binned_in_maps(np_inputs, geom, lists):
    """Host tables for the v4 binned program."""
    w0c, w1c, w2c, Dc, valid = (geom["w0c"], geom["w1c"], geom["w2c"],
                                geom["Dc"], geom["valid"])
    xs = ((np.arange(W, dtype=np.float64) + 0.5) / W * 2.0 - 1.0)
    ys = (1.0 - (np.arange(H, dtype=np.float64) + 0.5) / H * 2.0)
    dy = -1.0 / 64.0
    nbc = W // BC

    # per-face coefficient stacks [3(basis rows A,B,C), F+1] with poison row
    def ext(c):
        z = np.zeros((3, F + 1))
        z[:, :F] = c
        return z
    e0, e1, e2, ed = ext(w0c), ext(w1c), ext(w2c), ext(Dc)

    # tanh'd texture cube, c-major, bf16 [F+1, 192]
    cube = np.tanh(np.asarray(np_inputs["textures"][0], np.float64))
    cube = cube.reshape(F, TS, TS, TS, 3).transpose(0, 4, 1, 2, 3)
    cube_ext = np.zeros((F + 1, CUBE), np.float32)
    cube_ext[:F] = cube.reshape(F, CUBE)
    cube_bf = cube_ext.astype(ml_dtypes.bfloat16)

    pvalid = np.concatenate([valid, [False]])
    image_ref = np.asarray(np_inputs["image_ref"])

    # constant pixel basis, replicated into PE row groups 0/32/64/96
    j = np.arange(128) % 64
    r = (np.arange(128) // 64).astype(np.float64)
    xs0 = xs[j]
    pb3 = np.stack([np.ones(128), xs0, r]).astype(np.float32)  # [3,128]
    pb = np.zeros((12, 128), np.float32)
    for g in range(4):
        pb[3 * g:3 * g + 3] = pb3
    kk = np.broadcast_to(
        np.arange(4, dtype=np.float32),
        (128, NBLK * 3, 4)).reshape(128, NBLK * 12).copy()
    ident = np.eye(128, dtype=np.float32)

    in_maps = []
    for c in range(NCORES):
        # sort blocks dense-first so window-1 real faces fit slots 0..39
        cnt = (lists[c] < F).sum(axis=1)
        perm = np.argsort(-cnt, kind="stable")
        assert np.sort(cnt)[::-1][WB:].max() <= 40
        li = lists[c][perm]                            # [NBLK, CAP]
        # per-block folded affine: A' = A + B*bj + C*py0 over basis (1,xs0,r)
        blkrow = perm // nbc
        bj = (perm % nbc).astype(np.float64)
        py0 = ys[c * TPC + blkrow * BR]
        rcb = np.zeros((3, NBLK, CAP, 4))
        cf = np.zeros((NBLK, CAP, CROW))
        for e_i, e in enumerate((e0, e1, e2)):
            A = e[0][li]; B = e[1][li]; C = e[2][li]   # [NBLK, CAP]
            Af = A + B * bj[:, None] + C * py0[:, None]
            rcb[0, :, :, e_i] = -KSCALE * Af
            rcb[1, :, :, e_i] = -KSCALE * B
            rcb[2, :, :, e_i] = -KSCALE * (C * dy)
            cf[:, :, 3 * e_i] = Af
            cf[:, :, 3 * e_i + 1] = B
            cf[:, :, 3 * e_i + 2] = C * dy
        A = ed[0][li]; B = ed[1][li]; C = ed[2][li]
        rcb[0, :, :, 3] = (A + B * bj[:, None] + C * py0[:, None]) + DSHIFT
        rcb[1, :, :, 3] = B
        rcb[2, :, :, 3] = C * dy
        # poison: padded slots and degenerate faces -> key 1e30
        poison = pvalid[li] == False                    # noqa: E712
        rcb[0][poison] = [1e30, 0.0, 0.0, 0.0]
        rcb[1][poison] = 0.0
        rcb[2][poison] = 0.0
        cf[poison] = 0.0
        rcb = rcb.reshape(3, NBLK * CAP * 4).astype(np.float32)
        rcbq = np.zeros((12, NBLK * CAP * 4), np.float32)
        for g in range(4):
            rcbq[3 * g:3 * g + 3] = rcb

        # block-diagonal slot table: block t's 64 slots live on partitions
        # 0-63 (even t) or 64-127 (odd t); the other half stays zero so a
        # full K=128 one-hot matmul selects without cross-block bleed.
        # row = [cube bf16 | coef_hi bf16 | coef_lo bf16 (exact split)]
        SROW = CUBE + 24
        tabc = np.zeros((128, NBLK * SROW), ml_dtypes.bfloat16)
        cf32 = cf.astype(np.float32)
        cf_hi = cf32.astype(ml_dtypes.bfloat16)
        cf_lo = (cf32 - cf_hi.astype(np.float32)).astype(ml_dtypes.bfloat16)
        for t in range(NBLK):
            b = t % 2
            rows = slice(64 * b, 64 * (b + 1))
            tabc[rows, t * SROW:t * SROW + CUBE] = cube_bf[li[t]]
            tabc[rows, t * SROW + CUBE:t * SROW + CUBE + 12] = cf_hi[t]
            tabc[rows, t * SROW + CUBE + 12:t * SROW + SROW] = cf_lo[t]

        refsl = np.zeros((128, NBLK, 3), np.float32)
        for t in range(NBLK):
            br_, bj_ = divmod(int(perm[t]), nbc)
            rows = c * TPC + br_ * BR + np.arange(BR)
            cols = bj_ * BC + np.arange(BC)
            refsl[:, t, :] = image_ref[0][:, rows, :][:, :, cols] \
                .transpose(1, 2, 0).reshape(128, 3)

        cpkd = np.concatenate([
            pb3[1:2].T.astype(np.float32),       # xs0
            pb3[2:3].T.astype(np.float32),       # rvec
            kk,                                  # tent k table
            refsl.reshape(128, NBLK * 3),        # reference slice
            ident,                               # transpose identity
        ], axis=1)
        in_maps.append({"rcb": rcbq, "pb": pb, "cpk": cpkd,
                        "tabc": tabc})
    return in_maps


_last_exec_ns = None
_last_results = None
_last_in_maps = None


def kernel(vertices=None, textures=None, image_ref=None, faces=None,
           _trace=False, _probes=False, **kw):
    global _last_exec_ns, _last_results, _last_in_maps
    from concourse.bass_utils import run_bass_kernel_spmd

    vertices = np.asarray(vertices)
    textures = np.asarray(textures)
    image_ref = np.asarray(image_ref)
    faces = np.asarray(faces)
    np_inputs = {"vertices": vertices, "textures": textures,
                 "image_ref": image_ref, "faces": faces}

    geom = _geom(vertices, faces)
    lists = _bin_faces(geom)
    assert lists is not None, "bin overflow: CAP too small for this input"
    in_maps = _binned_in_maps(np_inputs, geom, lists)
    key = ("nc", _probes)
    if key not in _prog_cache:
        _prog_cache[key] = _build_binned(probes=_probes)
    nc = _prog_cache[key]
    _last_in_maps = in_maps
    res = run_bass_kernel_spmd(nc, in_maps, core_ids=list(range(NCORES)),
                               trace=_trace)
    _last_exec_ns = res.exec_time_ns
    _last_results = res
    total = np.float32(0.0)
    for r in res.results:
        total += np.float32(r["lossp"].reshape(()))
    return np.asarray(total, np.float32)


# revision 56
# speedup vs baseline: 2.3877x; 2.3877x over previous
"""Trainium2 Bass kernel for the neural-renderer loss model (v5).

Pixels are sharded 16 image rows per core across 8 cores.  Each core's
2048 pixels are 16 blocks of 128 px (2 rows x 64 cols), 2 windows of 8.

v5 replaces the baseline's per-pixel dma_gather (descriptor-bound,
~17us/iter on GpSimd) with exact one-hot matmuls on the Tensor engine:

  1. Raster: per-(block,slot) affine coefficients over the constant pixel
     basis (1, xs0[j], r) -> 4 concurrent fp32r matmuls -> PSUM grids ->
     grouped negated max-reduces -> nk keys -> per-block nkmax.
  2. Winner one-hot WITHOUT max_index: ohp[p,(t,s)] = is_eq(nk, nkmax)
     in bf16.  Ties/empty blocks hit all-zero poison rows, so
     multi-matches are harmless.
  3. PE transposes ohp (bf16, 2 blocks per [128,128] slice) to
     slot-major; ScalarE copies PSUM->SBUF.
  4. One K=128 bf16 matmul per block PAIR selects texture cube + coefs
     from a block-diagonal zero-padded SBUF table (even block's slots on
     partitions 0-63, odd on 64-127; two concurrent K=64 row-group
     matmuls at tile_position (0,0)+(64,0) HANG the hardware - hence
     full-height with zero padding).  Coefs ride as exact bf16 hi/lo
     splits; one-hot weights keep all products exact.
  5. ScalarE copies coef then cube PSUM->SBUF; GpSimd reconstructs
     coef=hi+lo; DVE tail: barycentrics, clip/renorm, tent weights,
     separable w64, bf16 cube multiply + fold + grouped reduce, hit
     mask, diff vs reference.
  6. Loss: fused diff^2-accumulate (DVE) + GpSimd partition_all_reduce.

Scheduling: the slot table is bulk-DMA'd at t=0 (no dependencies);
rcb/pb DMA only the 12 real partitions.  The timing build unrolls the
hardware loop 8x with software pipelining (tails one body behind
fronts, deep-buffered tail constants) so DVE holes fill with neighbor
iterations' work; an ACT-table warmup before the loop avoids a
recurring 1.3us reload.
"""
import numpy as np
import ml_dtypes

H = W = 128
TS = 4
F = 2560
DIST, ELEV, AZIM = 2.732, 0.0, 90.0
NCORES = 8
TPC = H // NCORES            # image rows per core
KSCALE = 1e20
DSHIFT = 4.0                 # small shift keeps depth positive yet precise
HIT_THRESH = 1e6

BR, BC = 2, 64               # block shape (rows x cols), 128 px/block
NBLK = TPC * 128 // (BR * BC)   # 16 blocks per core
CAP = 64                     # face slots per block
NWIN = 2                     # windows (8 blocks each)
WB = NBLK // NWIN            # blocks per window
NPAIR = NBLK // 2            # block pairs (even on part 0-63, odd 64-127)
CUBE = 192                   # cube row (c-major 3*4*4*4)
CROW = 12                    # coef row (9 used + pad)

_prog_cache = {}


def _geom(vertices, faces):
    v64 = np.asarray(vertices[0], np.float64)
    el, az = np.deg2rad(ELEV), np.deg2rad(AZIM)
    eye = DIST * np.array(
        [np.cos(el) * np.sin(az), np.sin(el), -np.cos(el) * np.cos(az)]
    )
    up = np.array([0.0, 1.0, 0.0])
    z = -eye / np.linalg.norm(eye)
    x = np.cross(up, z); x = x / np.linalg.norm(x)
    y = np.cross(z, x)
    R = np.stack([x, y, z])
    vc = (v64 - eye) @ R.T
    tri = vc[np.asarray(faces[0])]               # [F,3,3]
    a, b, c = tri[:, 0], tri[:, 1], tri[:, 2]
    area = (b[:, 0] - a[:, 0]) * (c[:, 1] - a[:, 1]) - \
           (b[:, 1] - a[:, 1]) * (c[:, 0] - a[:, 0])
    sa = np.where(np.abs(area) < 1e-8, 1e-8, area)
    valid = np.abs(area) >= 1e-8

    def edge_coeffs(p, q):
        # edge(p,q,pt) = (qx-px)(pty-py) - (qy-py)(ptx-px) = A + B*ptx + C*pty
        A = p[:, 0] * q[:, 1] - p[:, 1] * q[:, 0]
        B = -(q[:, 1] - p[:, 1])
        C = q[:, 0] - p[:, 0]
        return np.stack([A, B, C])               # [3,F]

    w0c = edge_coeffs(b, c) / sa
    w1c = edge_coeffs(c, a) / sa
    w2c = edge_coeffs(a, b) / sa
    z3 = tri[:, :, 2]
    Dc = w0c * z3[:, 0] + w1c * z3[:, 1] + w2c * z3[:, 2]
    p2x = np.stack([a[:, 0], b[:, 0], c[:, 0]])
    p2y = np.stack([a[:, 1], b[:, 1], c[:, 1]])
    return dict(w0c=w0c, w1c=w1c, w2c=w2c, Dc=Dc, valid=valid,
                bbx=(p2x.min(0), p2x.max(0)), bby=(p2y.min(0), p2y.max(0)))


def _bin_faces(geom):
    """Per-(core, block) conservative face lists. None on CAP overflow."""
    xs = ((np.arange(W, dtype=np.float64) + 0.5) / W * 2.0 - 1.0)
    ys = (1.0 - (np.arange(H, dtype=np.float64) + 0.5) / H * 2.0)
    wcs = [geom["w0c"], geom["w1c"], geom["w2c"]]
    valid = geom["valid"]
    nbr, nbc = H // BR, W // BC
    lists = np.full((NCORES, NBLK, CAP), F, np.int64)   # pad = poison face F
    for bi in range(nbr):
        rcy = ys[bi * BR:(bi + 1) * BR]
        cy = (rcy[0] + rcy[-1]) / 2; hy = abs(rcy[-1] - rcy[0]) / 2
        for bj in range(nbc):
            rcx = xs[bj * BC:(bj + 1) * BC]
            cx = (rcx[0] + rcx[-1]) / 2; hx = (rcx[-1] - rcx[0]) / 2
            ok = valid.copy()
            bbx, bby = geom["bbx"], geom["bby"]
            ok &= (bbx[0] <= cx + hx + 1e-6) & (bbx[1] >= cx - hx - 1e-6)
            ok &= (bby[0] <= cy + hy + 1e-6) & (bby[1] >= cy - hy - 1e-6)
            for e in range(3):
                A, B, C = wcs[e][0], wcs[e][1], wcs[e][2]
                wmax = A + B * cx + C * cy + np.abs(B) * hx + np.abs(C) * hy
                eps = 1e-5 * (np.abs(A) + np.abs(B) + np.abs(C))
                ok &= (wmax + eps) >= 0
            idx = np.nonzero(ok)[0]
            if idx.size > CAP:
                # refine with the exact pixel-center test (+ fp slack)
                px = xs[bj * BC:(bj + 1) * BC]
                py = ys[bi * BR:(bi + 1) * BR]
                PY, PX = np.meshgrid(py, px, indexing="ij")
                P0, P1 = PX.ravel()[None, :], PY.ravel()[None, :]
                ins = np.ones((idx.size, BR * BC), bool)
                for e in range(3):
                    A = wcs[e][0][idx]; B = wcs[e][1][idx]; C = wcs[e][2][idx]
                    eps = 1e-5 * (np.abs(A) + np.abs(B) + np.abs(C))
                    w = A[:, None] + B[:, None] * P0 + C[:, None] * P1
                    ins &= (w + eps[:, None]) >= 0
                idx = idx[ins.any(1)]
                if idx.size > CAP:
                    return None
            core = (bi * BR) // TPC
            blkrow = bi - core * (TPC // BR)
            t = blkrow * nbc + bj
            lists[core, t, :idx.size] = idx
    return lists


def _build_binned(loop_n=None, probes=False, ablate=None):
    """Binned program v4. loop_n wraps the body in a hardware loop."""
    from contextlib import ExitStack
    import concourse.bacc as bacc
    import concourse.tile as tile
    from concourse import mybir
    from concourse._compat import axon_active

    fp32 = mybir.dt.float32
    fp32r = mybir.dt.float32r
    bf16 = mybir.dt.bfloat16
    AL = mybir.AluOpType
    nc = bacc.Bacc(
        "TRN2",
        target_bir_lowering=False,
        debug=not axon_active(),
        num_devices=NCORES,
    )

    GCOLS = CAP * 4                       # grid cols per block
    rcb_in = nc.dram_tensor("rcb", [3, 128 + NBLK * GCOLS], fp32r,
                            kind="ExternalInput").ap()
    SROW = CUBE + 24          # cube | coef_hi(12) | coef_lo(12), bf16
    ES = NPAIR * SROW         # even-blocks section size
    tabc_in = nc.dram_tensor("tabc", [64, 2 * ES], bf16,
                             kind="ExternalInput").ap()
    # packed small constants: [xs0 | rvec | kk4(4) | refsl(48) | identb(64)]
    NCONST = 2 + 4 + NBLK * 3 + 64
    cpk_in = nc.dram_tensor("cpk", [128, NCONST], fp32,
                            kind="ExternalInput").ap()
    lossp = nc.dram_tensor("lossp", [1, 1], fp32,
                           kind="ExternalOutput").ap()
    if probes:
        p_nkmax = nc.dram_tensor("p_nkmax", [128, NBLK], fp32,
                                 kind="ExternalOutput").ap()
        p_nk = nc.dram_tensor("p_nk", [128, NBLK * CAP], fp32,
                              kind="ExternalOutput").ap()
        p_oh = nc.dram_tensor("p_oh", [128, NWIN * WB * CAP], fp32,
                              kind="ExternalOutput").ap()
        p_cb = nc.dram_tensor("p_cb", [128, NBLK * CUBE], fp32,
                              kind="ExternalOutput").ap()
        p_diff = nc.dram_tensor("p_diff", [128, NBLK * 3], fp32,
                                kind="ExternalOutput").ap()
        p_acc = nc.dram_tensor("p_acc", [128, 1], fp32,
                               kind="ExternalOutput").ap()

    with tile.TileContext(nc) as tc, ExitStack() as ctx:
        const = ctx.enter_context(tc.tile_pool(name="const", bufs=2))
        sb = ctx.enter_context(tc.tile_pool(name="sb", bufs=3))
        sm = ctx.enter_context(tc.tile_pool(name="sm", bufs=3))
        smx = ctx.enter_context(tc.tile_pool(name="smx", bufs=4))
        cpt_p = ctx.enter_context(tc.tile_pool(name="cpt", bufs=4))
        cbp = ctx.enter_context(tc.tile_pool(name="cbp", bufs=3))
        psg = ctx.enter_context(tc.tile_pool(name="psg", bufs=2,
                                             space="PSUM"))
        pst = ctx.enter_context(tc.tile_pool(name="pst", bufs=2,
                                             space="PSUM"))
        psc = ctx.enter_context(tc.tile_pool(name="psc", bufs=1,
                                             space="PSUM"))
        from concourse import bass_isa as _bisa

        # warm the scalar-engine activation table before the loop so the
        # 1.3us ACT_TABLE_LOAD doesn't recur per iteration
        warm = sm.tile([128, 1], fp32, tag="warm")
        nc.vector.memset(warm[:], 0.0)
        wrm2 = sm.tile([128, 1], fp32, tag="warm2")
        nc.scalar.activation(wrm2[:], warm[:],
                             mybir.ActivationFunctionType.Copy)

        UNROLL = 8
        if loop_n is not None:
            assert loop_n % UNROLL == 0
            ctx.enter_context(tc.For_i(0, loop_n // UNROLL, 1,
                                         staggered_reset=True))

        def emit_front():
            st = {}
            # ---- inputs: only real partitions; raster operands first ----
            rcb_t = const.tile([3, 128 + NBLK * GCOLS], fp32r, tag="rcb")
            nc.sync.dma_start(out=rcb_t[:], in_=rcb_in[:])
            pb_t = rcb_t
            NTC = 6 + NBLK * 3
            cpt = cpt_p.tile([128, NCONST], fp32, tag="cpt")
            nc.scalar.dma_start(out=cpt[:], in_=cpk_in[:])
            st["xs0"] = cpt[:, 0:1]
            st["rvec"] = cpt[:, 1:2]
            st["kk"] = cpt[:, 2:6]
            st["rs"] = cpt[:, 6:6 + NBLK * 3].rearrange(
                "p (t c) -> p t c", c=3)
            ident = cpt[:, NTC:].bitcast(bf16)
            tabc = const.tile([128, 2 * ES], bf16, tag="tabc")
            # zero halves written on idle engines at alloc time; DMAs carry
            # only the real 64-partition halves
            nc.gpsimd.memset(tabc[64:128, 0:ES], 0.0)
            nc.scalar.activation(tabc[0:64, ES:2 * ES],
                                 cpt[0:64, 0:1].broadcast_to((64, ES)),
                                 mybir.ActivationFunctionType.Copy,
                                 scale=0.0)
            nc.sync.dma_start(out=tabc[0:64, 0:ES], in_=tabc_in[:, 0:ES])
            nc.scalar.dma_start(out=tabc[64:128, ES:2 * ES],
                                in_=tabc_in[:, ES:2 * ES])
            tabv = tabc[:].rearrange("p (h j c) -> p j h c", h=2, c=SROW)

            nk = const.tile([128, NBLK * CAP], fp32, tag="nk")
            nkmax = const.tile([128, NBLK], fp32, tag="nkmax")
            st["nk"] = nk
            st["nkmax"] = nkmax

            # window-1 sparse slots stay poisoned (host sorts dense-first)
            nc.gpsimd.memset(nk[:, WB * CAP:], -1e30)

            # ---- raster both windows ----
            ohps = []
            for w in range(NWIN):
                ws = slice(w * WB, (w + 1) * WB)
                wsl = slice(w * WB * CAP, (w + 1) * WB * CAP)
                for q in range(2):
                    pw = psg.tile([128, WB * GCOLS // 2], fp32, tag="grid")
                    for i in range(2):
                        g = 2 * q + i
                        t = w * WB + 2 * g
                        nc.tensor.matmul(
                            pw[:, i * 512:(i + 1) * 512],
                            lhsT=pb_t[0:3, 0:128],
                            rhs=rcb_t[0:3,
                                      128 + t * GCOLS:128 + (t + 2) * GCOLS],
                            start=True, stop=True,
                            tile_position=(0, 0),
                        )
                    hsl = slice((2 * w + q) * 4 * CAP,
                                (2 * w + q + 1) * 4 * CAP)
                    if w == 0 and q == 0:
                        # split the first reduce so DVE starts after quad 0
                        for u in range(2):
                            nc.vector.tensor_reduce(
                                nk[:, u * 2 * CAP:(u + 1) * 2 * CAP],
                                pw[:, u * 512:(u + 1) * 512].rearrange(
                                    "p (f v) -> p f v", v=4),
                                axis=mybir.AxisListType.X, op=AL.max,
                                negate=True)
                    elif w == 0:
                        nc.vector.tensor_reduce(
                            nk[:, hsl],
                            pw[:].rearrange("p (f v) -> p f v", v=4),
                            axis=mybir.AxisListType.X, op=AL.max,
                            negate=True)
                    else:
                        # sparse window: real faces sit in slots 0..39
                        nc.vector.tensor_reduce(
                            nk[:, hsl].rearrange("p (t s) -> p t s",
                                                 s=CAP)[:, :, 0:40],
                            pw[:].rearrange("p (t c) -> p t c",
                                            c=GCOLS)[:, :, 0:160]
                                .rearrange("p t (f v) -> p t f v", v=4),
                            axis=mybir.AxisListType.X, op=AL.max,
                            negate=True)
                nc.vector.tensor_reduce(
                    nkmax[:, ws],
                    nk[:, wsl].rearrange("p (t s) -> p t s", s=CAP)
                        [:, :, 0:(CAP if w == 0 else 40)],
                    axis=mybir.AxisListType.X, op=AL.max)
                # winner one-hot, pixel-major (exact fp32 compare)
                ohp = sm.tile([128, WB * CAP], bf16, tag="ohp")
                nc.vector.tensor_tensor(
                    ohp[:].rearrange("p (t s) -> p t s", s=CAP),
                    nk[:, wsl].rearrange("p (t s) -> p t s", s=CAP),
                    nkmax[:, ws].unsqueeze(2).broadcast_to((128, WB, CAP)),
                    op=AL.is_equal)
                ohps.append(ohp)

            st["ohps"] = ohps

            if ablate is not None:
                acca = sm.tile([128, 1], fp32, tag="acca")
                scr = sm.tile([128, NBLK], fp32, tag="scr")
                nc.vector.scalar_tensor_tensor(
                    scr[:], nkmax[:], 1.0, nkmax[:], op0=AL.mult,
                    op1=AL.mult, accum_out=acca[:])
                lsba = sm.tile([128, 1], fp32, tag="lsba")
                nc.gpsimd.partition_all_reduce(lsba[:], acca[:], channels=128,
                                               reduce_op=_bisa.ReduceOp.add)
                nc.sync.dma_start(out=lossp[:], in_=lsba[0:1, :])

            # ---- one-hot transpose + select, per window ----
            cubeb = cbp.tile([128, NBLK, CUBE], bf16, tag="cubeb")
            st["cubeb"] = cubeb
            ckhl = smx.tile([128, NBLK, 24], fp32, tag="ckhl")
            st["ckhl"] = ckhl
            for w in range(NWIN if ablate != "raster" else 0):
                ohp = ohps[w]
                psT = pst.tile([128, (WB // 2) * 128], bf16, tag="psT")
                for j in range(WB // 2):
                    nc.tensor.transpose(
                        psT[:, j * 128:(j + 1) * 128],
                        ohp[:, j * 128:(j + 1) * 128],
                        ident)
                ohb = sm.tile([128, (WB // 2) * 128], bf16, tag="ohb")
                nc.scalar.activation(ohb[:], psT[:],
                                     mybir.ActivationFunctionType.Copy)

                if ablate == "oh":
                    continue
                for h in range(2):
                    pc = psc.tile([128, 2, 512], fp32, tag="pc")
                    for jj in range(2):
                        j = 2 * h + jj          # pair within window
                        tg = w * WB + 2 * j     # first block of pair
                        nc.tensor.matmul(
                            pc[:, jj, 0:2 * SROW],
                            lhsT=ohb[:, j * 128:(j + 1) * 128],
                            rhs=tabv[:, w * (WB // 2) + j, :, :],
                            start=True, stop=True)
                    pcv = pc[:, :, 0:2 * SROW].rearrange(
                        "p j (b c) -> p j b c", b=2)
                    nc.scalar.activation(
                        ckhl[:, w * WB + 4 * h:w * WB + 4 * (h + 1), :]
                            .rearrange("p (j b) c -> p j b c", b=2),
                        pcv[:, :, :, CUBE:SROW],
                        mybir.ActivationFunctionType.Copy)
                    nc.scalar.activation(
                        cubeb[:, w * WB + 4 * h:w * WB + 4 * (h + 1), :]
                            .rearrange("p (j b) c -> p j b c", b=2),
                        pcv[:, :, :, 0:CUBE],
                        mybir.ActivationFunctionType.Copy)
            if ablate is None:
                ck = sm.tile([128, NBLK, CROW], fp32, tag="ck")
                nc.gpsimd.tensor_tensor(ck[:], ckhl[:, :, 0:CROW],
                                        ckhl[:, :, CROW:24], op=AL.add)
                st["ck"] = ck
            return st

        def emit_tail(st):
            if ablate is not None:
                return
            xs0, rvec, kk, rs = st["xs0"], st["rvec"], st["kk"], st["rs"]
            nkmax, cubeb = st["nkmax"], st["cubeb"]
            diff = const.tile([128, NBLK, 3], fp32, tag="diff")

            # ---- winner barycentric u_i = clip(A' + B*xs0 + C'*r) ----
            ck = st["ck"]
            Av = ck[:, :, 0:9:3]
            Bv = ck[:, :, 1:9:3]
            Cv = ck[:, :, 2:9:3]
            u3 = sm.tile([128, NBLK, 3], fp32, tag="u3")
            nc.vector.scalar_tensor_tensor(
                u3[:], Bv, xs0, Av, op0=AL.mult, op1=AL.add)
            nc.vector.scalar_tensor_tensor(
                u3[:], Cv, rvec, u3[:], op0=AL.mult, op1=AL.add)
            # lower clip at 1e-8 keeps ssum nonzero (folds the +eps op)
            nc.vector.tensor_scalar(u3[:], u3[:], 1e-8, 1.0, AL.max, AL.min)
            ssum = sm.tile([128, NBLK], fp32, tag="ssum")
            nc.vector.tensor_reduce(ssum[:], u3[:],
                                    axis=mybir.AxisListType.X, op=AL.add)
            rcp = sm.tile([128, NBLK], fp32, tag="rcp")
            nc.vector.reciprocal(rcp[:], ssum[:])
            q3 = sm.tile([128, NBLK, 3], fp32, tag="q3")
            nc.vector.tensor_tensor(
                q3[:], u3[:],
                rcp[:].unsqueeze(2).broadcast_to((128, NBLK, 3)),
                op=AL.mult)

            # ---- tents te = relu(min(1 - d, 1 + d)), d = 3q - k ----
            d4 = sb.tile([128, NBLK, 3, 4], fp32, tag="d4")
            te = sb.tile([128, NBLK, 3, 4], bf16, tag="te")
            nc.vector.scalar_tensor_tensor(
                d4[:].rearrange("p t c k -> p (t c) k"),
                q3[:].rearrange("p t c -> p (t c)").unsqueeze(2)
                    .broadcast_to((128, NBLK * 3, 4)),
                3.0, kk[:].unsqueeze(1).broadcast_to((128, NBLK * 3, 4)),
                op0=AL.mult, op1=AL.subtract)
            tw = sb.tile([128, NBLK, 3, 4], fp32, tag="tw")
            nc.vector.tensor_scalar(tw[:], d4[:], -1.0, 1.0,
                                    AL.mult, AL.add)
            nc.vector.tensor_scalar(d4[:], d4[:], 1.0, None, AL.add)
            nc.vector.tensor_tensor(d4[:], d4[:], tw[:], op=AL.min)
            nc.vector.tensor_scalar(te[:], d4[:], 0.0, None, AL.max)

            # ---- separable weights w64 = t0 x t1 x t2 ----
            w01 = sb.tile([128, NBLK, 4, 4], bf16, tag="w01")
            nc.vector.tensor_tensor(
                w01[:],
                te[:, :, 0, :].unsqueeze(3)
                    .broadcast_to((128, NBLK, 4, 4)),
                te[:, :, 1, :].unsqueeze(2)
                    .broadcast_to((128, NBLK, 4, 4)),
                op=AL.mult)
            w64 = sb.tile([128, NBLK, 16, 4], bf16, tag="w64")
            nc.vector.tensor_tensor(
                w64[:],
                w01[:].rearrange("p t a b -> p t (a b)").unsqueeze(3)
                    .broadcast_to((128, NBLK, 16, 4)),
                te[:, :, 2, :].unsqueeze(2)
                    .broadcast_to((128, NBLK, 16, 4)),
                op=AL.mult)

            # ---- cube contraction: bf16 multiply + fold + reduce ----
            mb = sb.tile([128, NBLK, 3, 64], bf16, tag="mb")
            nc.vector.tensor_tensor(
                mb[:],
                cubeb[:].rearrange("p t (c s) -> p t c s", s=64),
                w64[:].rearrange("p t a b -> p t (a b)").unsqueeze(2)
                    .broadcast_to((128, NBLK, 3, 64)),
                op=AL.mult)
            mbh = sb.tile([128, NBLK, 3, 32], bf16, tag="mbh")
            nc.vector.tensor_tensor(
                mbh[:], mb[:, :, :, 0:32],
                mb[:, :, :, 32:64], op=AL.add)
            col = sm.tile([128, NBLK, 3], fp32, tag="col")
            nc.vector.tensor_reduce(
                col[:], mbh[:].rearrange("p t c s -> p (t c) s"),
                axis=mybir.AxisListType.X, op=AL.add)

            # ---- hit mask + diff vs reference ----
            hm = sm.tile([128, NBLK], fp32, tag="hm")
            nc.vector.tensor_scalar(hm[:], nkmax[:], -HIT_THRESH,
                                    None, AL.is_gt)
            flat = sm.tile([128, NBLK, 3], fp32, tag="flat")
            nc.vector.tensor_tensor(
                flat[:], col[:],
                hm[:].unsqueeze(2).broadcast_to((128, NBLK, 3)),
                op=AL.mult)
            nc.vector.tensor_tensor(diff[:], flat[:], rs[:, :, :],
                                    op=AL.subtract)

            # ---- squared-error accumulate; host sums the 128 partitions ----
            sq = sb.tile([128, NBLK * 3], fp32, tag="sq")
            acc = sm.tile([128, 1], fp32, tag="acc")
            dv = diff[:].rearrange("p t c -> p (t c)")
            nc.vector.scalar_tensor_tensor(sq[:], dv, 1.0, dv, op0=AL.mult,
                                           op1=AL.mult, accum_out=acc[:])
            lsb = sm.tile([128, 1], fp32, tag="lsb")
            nc.gpsimd.partition_all_reduce(lsb[:], acc[:], channels=128,
                                           reduce_op=_bisa.ReduceOp.add)
            nc.sync.dma_start(out=lossp[:], in_=lsb[0:1, :])
            if probes:
                nc.sync.dma_start(out=p_nkmax[:], in_=st["nkmax"][:])
                nc.sync.dma_start(out=p_nk[:], in_=st["nk"][:])
                nc.sync.dma_start(out=p_cb[:],
                                  in_=cubeb[:].rearrange("p t c -> p (t c)"))
                nc.sync.dma_start(out=p_diff[:],
                                  in_=diff[:].rearrange("p t c -> p (t c)"))
                nc.sync.dma_start(out=p_acc[:], in_=acc[:])

        PIPE = True
        if loop_n is None or not PIPE:
            for _ in range(1 if loop_n is None else UNROLL):
                emit_tail(emit_front())
        else:
            prev = emit_front()
            for _ in range(UNROLL - 1):
                cur = emit_front()
                emit_tail(prev)
                prev = cur
            emit_tail(prev)

    nc.compile()
    return nc


def _binned_in_maps(np_inputs, geom, lists):
    """Host tables for the v4 binned program."""
    w0c, w1c, w2c, Dc, valid = (geom["w0c"], geom["w1c"], geom["w2c"],
                                geom["Dc"], geom["valid"])
    xs = ((np.arange(W, dtype=np.float64) + 0.5) / W * 2.0 - 1.0)
    ys = (1.0 - (np.arange(H, dtype=np.float64) + 0.5) / H * 2.0)
    dy = -1.0 / 64.0
    nbc = W // BC

    # per-face coefficient stacks [3(basis rows A,B,C), F+1] with poison row
    def ext(c):
        z = np.zeros((3, F + 1))
        z[:, :F] = c
        return z
    e0, e1, e2, ed = ext(w0c), ext(w1c), ext(w2c), ext(Dc)

    # tanh'd texture cube, c-major, bf16 [F+1, 192]
    cube = np.tanh(np.asarray(np_inputs["textures"][0], np.float64))
    cube = cube.reshape(F, TS, TS, TS, 3).transpose(0, 4, 1, 2, 3)
    cube_ext = np.zeros((F + 1, CUBE), np.float32)
    cube_ext[:F] = cube.reshape(F, CUBE)
    cube_bf = cube_ext.astype(ml_dtypes.bfloat16)

    pvalid = np.concatenate([valid, [False]])
    image_ref = np.asarray(np_inputs["image_ref"])

    # constant pixel basis, replicated into PE row groups 0/32/64/96
    j = np.arange(128) % 64
    r = (np.arange(128) // 64).astype(np.float64)
    xs0 = xs[j]
    pb3 = np.stack([np.ones(128), xs0, r]).astype(np.float32)  # [3,128]
    pb = np.zeros((12, 128), np.float32)
    for g in range(4):
        pb[3 * g:3 * g + 3] = pb3
    kk = np.broadcast_to(
        np.arange(4, dtype=np.float32), (128, 4)).copy()
    ident = np.eye(128, dtype=np.float32)
    identb = np.eye(128, dtype=ml_dtypes.bfloat16).view(np.uint16)
    identb = identb.reshape(128, 64, 2).astype(np.uint32)
    identb = (identb[:, :, 0] | (identb[:, :, 1] << 16)).view(np.float32)

    in_maps = []
    for c in range(NCORES):
        # sort blocks dense-first so window-1 real faces fit slots 0..39
        cnt = (lists[c] < F).sum(axis=1)
        perm = np.argsort(-cnt, kind="stable")
        assert np.sort(cnt)[::-1][WB:].max() <= 40
        li = lists[c][perm]                            # [NBLK, CAP]
        # per-block folded affine: A' = A + B*bj + C*py0 over basis (1,xs0,r)
        blkrow = perm // nbc
        bj = (perm % nbc).astype(np.float64)
        py0 = ys[c * TPC + blkrow * BR]
        rcb = np.zeros((3, NBLK, CAP, 4))
        cf = np.zeros((NBLK, CAP, CROW))
        for e_i, e in enumerate((e0, e1, e2)):
            A = e[0][li]; B = e[1][li]; C = e[2][li]   # [NBLK, CAP]
            Af = A + B * bj[:, None] + C * py0[:, None]
            rcb[0, :, :, e_i] = -KSCALE * Af
            rcb[1, :, :, e_i] = -KSCALE * B
            rcb[2, :, :, e_i] = -KSCALE * (C * dy)
            cf[:, :, 3 * e_i] = Af
            cf[:, :, 3 * e_i + 1] = B
            cf[:, :, 3 * e_i + 2] = C * dy
        A = ed[0][li]; B = ed[1][li]; C = ed[2][li]
        rcb[0, :, :, 3] = (A + B * bj[:, None] + C * py0[:, None]) + DSHIFT
        rcb[1, :, :, 3] = B
        rcb[2, :, :, 3] = C * dy
        # poison: padded slots and degenerate faces -> key 1e30
        poison = pvalid[li] == False                    # noqa: E712
        rcb[0][poison] = [1e30, 0.0, 0.0, 0.0]
        rcb[1][poison] = 0.0
        rcb[2][poison] = 0.0
        cf[poison] = 0.0
        rcb = rcb.reshape(3, NBLK * CAP * 4).astype(np.float32)
        rcbq = np.zeros((3, 128 + NBLK * CAP * 4), np.float32)
        rcbq[:, 0:128] = pb3
        rcbq[:, 128:] = rcb

        # block-diagonal slot table in section layout: even block t=2j on
        # partitions 0-63 at col j*SROW, odd t=2j+1 on partitions 64-127 at
        # ES + j*SROW; other halves zero so a full K=128 one-hot matmul
        # selects without cross-block bleed.
        # row = [cube bf16 | coef_hi bf16 | coef_lo bf16 (exact split)]
        SROW = CUBE + 24
        ES = (NBLK // 2) * SROW
        tabc = np.zeros((64, 2 * ES), ml_dtypes.bfloat16)
        cf32 = cf.astype(np.float32)
        cf_hi = cf32.astype(ml_dtypes.bfloat16)
        cf_lo = (cf32 - cf_hi.astype(np.float32)).astype(ml_dtypes.bfloat16)
        for t in range(NBLK):
            base = (t % 2) * ES + (t // 2) * SROW
            tabc[:, base:base + CUBE] = cube_bf[li[t]]
            tabc[:, base + CUBE:base + CUBE + 12] = cf_hi[t]
            tabc[:, base + CUBE + 12:base + SROW] = cf_lo[t]

        refsl = np.zeros((128, NBLK, 3), np.float32)
        for t in range(NBLK):
            br_, bj_ = divmod(int(perm[t]), nbc)
            rows = c * TPC + br_ * BR + np.arange(BR)
            cols = bj_ * BC + np.arange(BC)
            refsl[:, t, :] = image_ref[0][:, rows, :][:, :, cols] \
                .transpose(1, 2, 0).reshape(128, 3)

        cpkd = np.concatenate([
            pb3[1:2].T.astype(np.float32),       # xs0
            pb3[2:3].T.astype(np.float32),       # rvec
            kk,                                  # tent k table
            refsl.reshape(128, NBLK * 3),        # reference slice
            identb,                              # bf16 transpose identity
        ], axis=1)
        in_maps.append({"rcb": rcbq, "cpk": cpkd, "tabc": tabc})
    return in_maps


_last_exec_ns = None
_last_results = None
_last_in_maps = None


def kernel(vertices=None, textures=None, image_ref=None, faces=None,
           _trace=False, _probes=False, **kw):
    global _last_exec_ns, _last_results, _last_in_maps
    from concourse.bass_utils import run_bass_kernel_spmd

    vertices = np.asarray(vertices)
    textures = np.asarray(textures)
    image_ref = np.asarray(image_ref)
    faces = np.asarray(faces)
    np_inputs = {"vertices": vertices, "textures": textures,
                 "image_ref": image_ref, "faces": faces}

    geom = _geom(vertices, faces)
    lists = _bin_faces(geom)
    assert lists is not None, "bin overflow: CAP too small for this input"
    in_maps = _binned_in_maps(np_inputs, geom, lists)
    key = ("nc", _probes)
    if key not in _prog_cache:
        _prog_cache[key] = _build_binned(probes=_probes)
    nc = _prog_cache[key]
    _last_in_maps = in_maps
    res = run_bass_kernel_spmd(nc, in_maps, core_ids=list(range(NCORES)),
                               trace=_trace)
    _last_exec_ns = res.exec_time_ns
    _last_results = res
    total = np.float32(0.0)
    for r in res.results:
        total += np.float32(r["lossp"].reshape(()))
    return np.asarray(total, np.float32)


# revision 57
# speedup vs baseline: 3.1157x; 1.3049x over previous
"""Trainium2 Bass kernel for the neural-renderer loss model (v5).

Pixels are sharded 16 image rows per core across 8 cores.  Each core's
2048 pixels are 16 blocks of 128 px (2 rows x 64 cols), 2 windows of 8.

v5 replaces the baseline's per-pixel dma_gather (descriptor-bound,
~17us/iter on GpSimd) with exact one-hot matmuls on the Tensor engine:

  1. Raster: per-(block,slot) affine coefficients over the constant pixel
     basis (1, xs0[j], r) -> 4 concurrent fp32r matmuls -> PSUM grids ->
     grouped negated max-reduces -> nk keys -> per-block nkmax.
  2. Winner one-hot WITHOUT max_index: ohp[p,(t,s)] = is_eq(nk, nkmax)
     in bf16.  Ties/empty blocks hit all-zero poison rows, so
     multi-matches are harmless.
  3. PE transposes ohp (bf16, 2 blocks per [128,128] slice) to
     slot-major; ScalarE copies PSUM->SBUF.
  4. One K=128 bf16 matmul per block PAIR selects texture cube + coefs
     from a block-diagonal zero-padded SBUF table (even block's slots on
     partitions 0-63, odd on 64-127; two concurrent K=64 row-group
     matmuls at tile_position (0,0)+(64,0) HANG the hardware - hence
     full-height with zero padding).  Coefs ride as exact bf16 hi/lo
     splits; one-hot weights keep all products exact.
  5. ScalarE copies coef then cube PSUM->SBUF; GpSimd reconstructs
     coef=hi+lo; DVE tail: barycentrics, clip/renorm, tent weights,
     separable w64, bf16 cube multiply + fold + grouped reduce, hit
     mask, diff vs reference.
  6. Loss: fused diff^2-accumulate (DVE) + GpSimd partition_all_reduce.

Scheduling: the slot table is bulk-DMA'd at t=0 (no dependencies);
rcb/pb DMA only the 12 real partitions.  The timing build unrolls the
hardware loop 8x with software pipelining (tails one body behind
fronts, deep-buffered tail constants) so DVE holes fill with neighbor
iterations' work; an ACT-table warmup before the loop avoids a
recurring 1.3us reload.
"""
import numpy as np
import ml_dtypes

H = W = 128
TS = 4
F = 2560
DIST, ELEV, AZIM = 2.732, 0.0, 90.0
NCORES = 8
TPC = H // NCORES            # image rows per core
KSCALE = 1e20
DSHIFT = 4.0                 # small shift keeps depth positive yet precise
HIT_THRESH = 1e6

BR, BC = 2, 64               # block shape (rows x cols), 128 px/block
NBLK = TPC * 128 // (BR * BC)   # 16 blocks per core
CAP = 64                     # face slots per block
NWIN = 2                     # windows (8 blocks each)
WB = NBLK // NWIN            # blocks per window
NPAIR = NBLK // 2            # block pairs (even on part 0-63, odd 64-127)
CUBE = 192                   # cube row (c-major 3*4*4*4)
CROW = 12                    # coef row (9 used + pad)

_prog_cache = {}


def _geom(vertices, faces):
    v64 = np.asarray(vertices[0], np.float64)
    el, az = np.deg2rad(ELEV), np.deg2rad(AZIM)
    eye = DIST * np.array(
        [np.cos(el) * np.sin(az), np.sin(el), -np.cos(el) * np.cos(az)]
    )
    up = np.array([0.0, 1.0, 0.0])
    z = -eye / np.linalg.norm(eye)
    x = np.cross(up, z); x = x / np.linalg.norm(x)
    y = np.cross(z, x)
    R = np.stack([x, y, z])
    vc = (v64 - eye) @ R.T
    tri = vc[np.asarray(faces[0])]               # [F,3,3]
    a, b, c = tri[:, 0], tri[:, 1], tri[:, 2]
    area = (b[:, 0] - a[:, 0]) * (c[:, 1] - a[:, 1]) - \
           (b[:, 1] - a[:, 1]) * (c[:, 0] - a[:, 0])
    sa = np.where(np.abs(area) < 1e-8, 1e-8, area)
    valid = np.abs(area) >= 1e-8

    def edge_coeffs(p, q):
        # edge(p,q,pt) = (qx-px)(pty-py) - (qy-py)(ptx-px) = A + B*ptx + C*pty
        A = p[:, 0] * q[:, 1] - p[:, 1] * q[:, 0]
        B = -(q[:, 1] - p[:, 1])
        C = q[:, 0] - p[:, 0]
        return np.stack([A, B, C])               # [3,F]

    w0c = edge_coeffs(b, c) / sa
    w1c = edge_coeffs(c, a) / sa
    w2c = edge_coeffs(a, b) / sa
    z3 = tri[:, :, 2]
    Dc = w0c * z3[:, 0] + w1c * z3[:, 1] + w2c * z3[:, 2]
    p2x = np.stack([a[:, 0], b[:, 0], c[:, 0]])
    p2y = np.stack([a[:, 1], b[:, 1], c[:, 1]])
    return dict(w0c=w0c, w1c=w1c, w2c=w2c, Dc=Dc, valid=valid,
                bbx=(p2x.min(0), p2x.max(0)), bby=(p2y.min(0), p2y.max(0)))


def _bin_faces(geom):
    """Per-(core, block) conservative face lists. None on CAP overflow."""
    xs = ((np.arange(W, dtype=np.float64) + 0.5) / W * 2.0 - 1.0)
    ys = (1.0 - (np.arange(H, dtype=np.float64) + 0.5) / H * 2.0)
    wcs = [geom["w0c"], geom["w1c"], geom["w2c"]]
    valid = geom["valid"]
    nbr, nbc = H // BR, W // BC
    lists = np.full((NCORES, NBLK, CAP), F, np.int64)   # pad = poison face F
    for bi in range(nbr):
        rcy = ys[bi * BR:(bi + 1) * BR]
        cy = (rcy[0] + rcy[-1]) / 2; hy = abs(rcy[-1] - rcy[0]) / 2
        for bj in range(nbc):
            rcx = xs[bj * BC:(bj + 1) * BC]
            cx = (rcx[0] + rcx[-1]) / 2; hx = (rcx[-1] - rcx[0]) / 2
            ok = valid.copy()
            bbx, bby = geom["bbx"], geom["bby"]
            ok &= (bbx[0] <= cx + hx + 1e-6) & (bbx[1] >= cx - hx - 1e-6)
            ok &= (bby[0] <= cy + hy + 1e-6) & (bby[1] >= cy - hy - 1e-6)
            for e in range(3):
                A, B, C = wcs[e][0], wcs[e][1], wcs[e][2]
                wmax = A + B * cx + C * cy + np.abs(B) * hx + np.abs(C) * hy
                eps = 1e-5 * (np.abs(A) + np.abs(B) + np.abs(C))
                ok &= (wmax + eps) >= 0
            idx = np.nonzero(ok)[0]
            if idx.size > CAP:
                # refine with the exact pixel-center test (+ fp slack)
                px = xs[bj * BC:(bj + 1) * BC]
                py = ys[bi * BR:(bi + 1) * BR]
                PY, PX = np.meshgrid(py, px, indexing="ij")
                P0, P1 = PX.ravel()[None, :], PY.ravel()[None, :]
                ins = np.ones((idx.size, BR * BC), bool)
                for e in range(3):
                    A = wcs[e][0][idx]; B = wcs[e][1][idx]; C = wcs[e][2][idx]
                    eps = 1e-5 * (np.abs(A) + np.abs(B) + np.abs(C))
                    w = A[:, None] + B[:, None] * P0 + C[:, None] * P1
                    ins &= (w + eps[:, None]) >= 0
                idx = idx[ins.any(1)]
                if idx.size > CAP:
                    return None
            core = (bi * BR) // TPC
            blkrow = bi - core * (TPC // BR)
            t = blkrow * nbc + bj
            lists[core, t, :idx.size] = idx
    return lists


def _build_binned(loop_n=None, probes=False, ablate=None):
    """Binned program v4. loop_n wraps the body in a hardware loop."""
    from contextlib import ExitStack
    import concourse.bacc as bacc
    import concourse.tile as tile
    from concourse import mybir
    from concourse._compat import axon_active

    fp32 = mybir.dt.float32
    fp32r = mybir.dt.float32r
    bf16 = mybir.dt.bfloat16
    AL = mybir.AluOpType
    nc = bacc.Bacc(
        "TRN2",
        target_bir_lowering=False,
        debug=not axon_active(),
        num_devices=NCORES,
    )

    GCOLS = CAP * 4                       # grid cols per block
    rcb_in = nc.dram_tensor("rcb", [3, 128 + NBLK * GCOLS], fp32r,
                            kind="ExternalInput").ap()
    SROW = CUBE + 24          # cube | coef_hi(12) | coef_lo(12), bf16
    ES = NPAIR * SROW         # even-blocks section size
    tabc_in = nc.dram_tensor("tabc", [64, 2 * ES], bf16,
                             kind="ExternalInput").ap()
    # packed small constants: [xs0 | rvec | kk4(4) | refsl(48) | identb(64)]
    NCONST = 2 + 4 + NBLK * 3 + 64
    cpk_in = nc.dram_tensor("cpk", [128, NCONST], fp32,
                            kind="ExternalInput").ap()
    lossp = nc.dram_tensor("lossp", [1, 1], fp32,
                           kind="ExternalOutput").ap()
    if probes:
        p_nkmax = nc.dram_tensor("p_nkmax", [128, NBLK], fp32,
                                 kind="ExternalOutput").ap()
        p_nk = nc.dram_tensor("p_nk", [128, NBLK * CAP], fp32,
                              kind="ExternalOutput").ap()
        p_oh = nc.dram_tensor("p_oh", [128, NWIN * WB * CAP], fp32,
                              kind="ExternalOutput").ap()
        p_cb = nc.dram_tensor("p_cb", [128, NBLK * CUBE], fp32,
                              kind="ExternalOutput").ap()
        p_diff = nc.dram_tensor("p_diff", [128, NBLK * 3], fp32,
                                kind="ExternalOutput").ap()
        p_acc = nc.dram_tensor("p_acc", [128, 1], fp32,
                               kind="ExternalOutput").ap()

    with tile.TileContext(nc) as tc, ExitStack() as ctx:
        const = ctx.enter_context(tc.tile_pool(name="const", bufs=2))
        sb = ctx.enter_context(tc.tile_pool(name="sb", bufs=3))
        sm = ctx.enter_context(tc.tile_pool(name="sm", bufs=3))
        smx = ctx.enter_context(tc.tile_pool(name="smx", bufs=4))
        cpt_p = ctx.enter_context(tc.tile_pool(name="cpt", bufs=4))
        cbp = ctx.enter_context(tc.tile_pool(name="cbp", bufs=3))
        psg = ctx.enter_context(tc.tile_pool(name="psg", bufs=2,
                                             space="PSUM"))
        pst = ctx.enter_context(tc.tile_pool(name="pst", bufs=2,
                                             space="PSUM"))
        psc = ctx.enter_context(tc.tile_pool(name="psc", bufs=1,
                                             space="PSUM"))
        from concourse import bass_isa as _bisa

        # warm the scalar-engine activation table before the loop so the
        # 1.3us ACT_TABLE_LOAD doesn't recur per iteration
        warm = sm.tile([128, 1], fp32, tag="warm")
        nc.vector.memset(warm[:], 0.0)
        wrm2 = sm.tile([128, 1], fp32, tag="warm2")
        nc.scalar.activation(wrm2[:], warm[:],
                             mybir.ActivationFunctionType.Copy)

        UNROLL = 16
        if loop_n is not None:
            assert loop_n % UNROLL == 0
            ctx.enter_context(tc.For_i(0, loop_n // UNROLL, 1,
                                         staggered_reset=True))

        def emit_front():
            st = {}
            # ---- inputs: only real partitions; raster operands first ----
            rcb_t = const.tile([3, 128 + NBLK * GCOLS], fp32r, tag="rcb")
            nc.sync.dma_start(out=rcb_t[:], in_=rcb_in[:])
            pb_t = rcb_t
            NTC = 6 + NBLK * 3
            cpt = cpt_p.tile([128, NCONST], fp32, tag="cpt")
            nc.scalar.dma_start(out=cpt[:], in_=cpk_in[:])
            st["xs0"] = cpt[:, 0:1]
            st["rvec"] = cpt[:, 1:2]
            st["kk"] = cpt[:, 2:6]
            st["rs"] = cpt[:, 6:6 + NBLK * 3].rearrange(
                "p (t c) -> p t c", c=3)
            ident = cpt[:, NTC:].bitcast(bf16)
            tabc = const.tile([128, 2 * ES], bf16, tag="tabc")
            # zero halves written on idle engines at alloc time; DMAs carry
            # only the real 64-partition halves
            nc.gpsimd.memset(tabc[64:128, 0:ES], 0.0)
            nc.scalar.activation(tabc[0:64, ES:2 * ES],
                                 cpt[0:64, 0:1].broadcast_to((64, ES)),
                                 mybir.ActivationFunctionType.Copy,
                                 scale=0.0)
            nc.sync.dma_start(out=tabc[0:64, 0:ES], in_=tabc_in[:, 0:ES])
            nc.scalar.dma_start(out=tabc[64:128, ES:2 * ES],
                                in_=tabc_in[:, ES:2 * ES])
            tabv = tabc[:].rearrange("p (h j c) -> p j h c", h=2, c=SROW)

            nk = const.tile([128, NBLK * CAP], fp32, tag="nk")
            nkmax = const.tile([128, NBLK], fp32, tag="nkmax")
            st["nk"] = nk
            st["nkmax"] = nkmax

            # window-1 sparse slots stay poisoned (host sorts dense-first)
            nc.gpsimd.memset(nk[:, WB * CAP:], -1e30)

            # ---- raster both windows ----
            ohps = []
            for w in range(NWIN):
                ws = slice(w * WB, (w + 1) * WB)
                wsl = slice(w * WB * CAP, (w + 1) * WB * CAP)
                for q in range(2):
                    pw = psg.tile([128, WB * GCOLS // 2], fp32, tag="grid")
                    for i in range(2):
                        g = 2 * q + i
                        t = w * WB + 2 * g
                        nc.tensor.matmul(
                            pw[:, i * 512:(i + 1) * 512],
                            lhsT=pb_t[0:3, 0:128],
                            rhs=rcb_t[0:3,
                                      128 + t * GCOLS:128 + (t + 2) * GCOLS],
                            start=True, stop=True,
                            tile_position=(0, 0),
                        )
                    hsl = slice((2 * w + q) * 4 * CAP,
                                (2 * w + q + 1) * 4 * CAP)
                    if w == 0 and q == 0:
                        # split the first reduce so DVE starts after quad 0
                        for u in range(2):
                            nc.vector.tensor_reduce(
                                nk[:, u * 2 * CAP:(u + 1) * 2 * CAP],
                                pw[:, u * 512:(u + 1) * 512].rearrange(
                                    "p (f v) -> p f v", v=4),
                                axis=mybir.AxisListType.X, op=AL.max,
                                negate=True)
                    elif w == 0:
                        nc.vector.tensor_reduce(
                            nk[:, hsl],
                            pw[:].rearrange("p (f v) -> p f v", v=4),
                            axis=mybir.AxisListType.X, op=AL.max,
                            negate=True)
                    else:
                        # sparse window: real faces sit in slots 0..39
                        nc.vector.tensor_reduce(
                            nk[:, hsl].rearrange("p (t s) -> p t s",
                                                 s=CAP)[:, :, 0:40],
                            pw[:].rearrange("p (t c) -> p t c",
                                            c=GCOLS)[:, :, 0:160]
                                .rearrange("p t (f v) -> p t f v", v=4),
                            axis=mybir.AxisListType.X, op=AL.max,
                            negate=True)
                nc.vector.tensor_reduce(
                    nkmax[:, ws],
                    nk[:, wsl].rearrange("p (t s) -> p t s", s=CAP)
                        [:, :, 0:(CAP if w == 0 else 40)],
                    axis=mybir.AxisListType.X, op=AL.max)
                # winner one-hot, pixel-major (exact fp32 compare)
                ohp = sm.tile([128, WB * CAP], bf16, tag="ohp")
                nc.vector.tensor_tensor(
                    ohp[:].rearrange("p (t s) -> p t s", s=CAP),
                    nk[:, wsl].rearrange("p (t s) -> p t s", s=CAP),
                    nkmax[:, ws].unsqueeze(2).broadcast_to((128, WB, CAP)),
                    op=AL.is_equal)
                ohps.append(ohp)

            st["ohps"] = ohps

            if ablate is not None:
                acca = sm.tile([128, 1], fp32, tag="acca")
                scr = sm.tile([128, NBLK], fp32, tag="scr")
                nc.vector.scalar_tensor_tensor(
                    scr[:], nkmax[:], 1.0, nkmax[:], op0=AL.mult,
                    op1=AL.mult, accum_out=acca[:])
                lsba = sm.tile([128, 1], fp32, tag="lsba")
                nc.gpsimd.partition_all_reduce(lsba[:], acca[:], channels=128,
                                               reduce_op=_bisa.ReduceOp.add)
                nc.sync.dma_start(out=lossp[:], in_=lsba[0:1, :])

            # ---- one-hot transpose + select, per window ----
            cubeb = cbp.tile([128, NBLK, CUBE], bf16, tag="cubeb")
            st["cubeb"] = cubeb
            ckhl = smx.tile([128, NBLK, 24], fp32, tag="ckhl")
            st["ckhl"] = ckhl
            for w in range(NWIN if ablate != "raster" else 0):
                ohp = ohps[w]
                psT = pst.tile([128, (WB // 2) * 128], bf16, tag="psT")
                for j in range(WB // 2):
                    nc.tensor.transpose(
                        psT[:, j * 128:(j + 1) * 128],
                        ohp[:, j * 128:(j + 1) * 128],
                        ident)
                ohb = sm.tile([128, (WB // 2) * 128], bf16, tag="ohb")
                nc.scalar.activation(ohb[:], psT[:],
                                     mybir.ActivationFunctionType.Copy)

                if ablate == "oh":
                    continue
                for h in range(2):
                    pc = psc.tile([128, 2, 512], fp32, tag="pc")
                    for jj in range(2):
                        j = 2 * h + jj          # pair within window
                        tg = w * WB + 2 * j     # first block of pair
                        nc.tensor.matmul(
                            pc[:, jj, 0:2 * SROW],
                            lhsT=ohb[:, j * 128:(j + 1) * 128],
                            rhs=tabv[:, w * (WB // 2) + j, :, :],
                            start=True, stop=True)
                    pcv = pc[:, :, 0:2 * SROW].rearrange(
                        "p j (b c) -> p j b c", b=2)
                    nc.scalar.activation(
                        ckhl[:, w * WB + 4 * h:w * WB + 4 * (h + 1), :]
                            .rearrange("p (j b) c -> p j b c", b=2),
                        pcv[:, :, :, CUBE:SROW],
                        mybir.ActivationFunctionType.Copy)
                    nc.scalar.activation(
                        cubeb[:, w * WB + 4 * h:w * WB + 4 * (h + 1), :]
                            .rearrange("p (j b) c -> p j b c", b=2),
                        pcv[:, :, :, 0:CUBE],
                        mybir.ActivationFunctionType.Copy)
            if ablate is None:
                ck = sm.tile([128, NBLK, CROW], fp32, tag="ck")
                nc.gpsimd.tensor_tensor(ck[:], ckhl[:, :, 0:CROW],
                                        ckhl[:, :, CROW:24], op=AL.add)
                st["ck"] = ck
            return st

        def emit_tail(st):
            if ablate is not None:
                return
            xs0, rvec, kk, rs = st["xs0"], st["rvec"], st["kk"], st["rs"]
            nkmax, cubeb = st["nkmax"], st["cubeb"]
            diff = const.tile([128, NBLK, 3], fp32, tag="diff")

            # ---- winner barycentric u_i = clip(A' + B*xs0 + C'*r) ----
            ck = st["ck"]
            Av = ck[:, :, 0:9:3]
            Bv = ck[:, :, 1:9:3]
            Cv = ck[:, :, 2:9:3]
            u3 = sm.tile([128, NBLK, 3], fp32, tag="u3")
            nc.vector.scalar_tensor_tensor(
                u3[:], Bv, xs0, Av, op0=AL.mult, op1=AL.add)
            nc.vector.scalar_tensor_tensor(
                u3[:], Cv, rvec, u3[:], op0=AL.mult, op1=AL.add)
            # lower clip at 1e-8 keeps ssum nonzero (folds the +eps op)
            nc.vector.tensor_scalar(u3[:], u3[:], 1e-8, 1.0, AL.max, AL.min)
            ssum = sm.tile([128, NBLK], fp32, tag="ssum")
            nc.vector.tensor_reduce(ssum[:], u3[:],
                                    axis=mybir.AxisListType.X, op=AL.add)
            rcp = sm.tile([128, NBLK], fp32, tag="rcp")
            nc.vector.reciprocal(rcp[:], ssum[:])
            q3 = sm.tile([128, NBLK, 3], fp32, tag="q3")
            nc.vector.tensor_tensor(
                q3[:], u3[:],
                rcp[:].unsqueeze(2).broadcast_to((128, NBLK, 3)),
                op=AL.mult)

            # ---- tents te = relu(min(1 - d, 1 + d)), d = 3q - k ----
            d4 = sb.tile([128, NBLK, 3, 4], fp32, tag="d4")
            te = sb.tile([128, NBLK, 3, 4], bf16, tag="te")
            nc.vector.scalar_tensor_tensor(
                d4[:].rearrange("p t c k -> p (t c) k"),
                q3[:].rearrange("p t c -> p (t c)").unsqueeze(2)
                    .broadcast_to((128, NBLK * 3, 4)),
                3.0, kk[:].unsqueeze(1).broadcast_to((128, NBLK * 3, 4)),
                op0=AL.mult, op1=AL.subtract)
            tw = sb.tile([128, NBLK, 3, 4], fp32, tag="tw")
            nc.vector.tensor_scalar(tw[:], d4[:], -1.0, 1.0,
                                    AL.mult, AL.add)
            nc.vector.tensor_scalar(d4[:], d4[:], 1.0, None, AL.add)
            nc.vector.tensor_tensor(d4[:], d4[:], tw[:], op=AL.min)
            nc.vector.tensor_scalar(te[:], d4[:], 0.0, None, AL.max)

            # ---- separable weights w64 = t0 x t1 x t2 ----
            w01 = sb.tile([128, NBLK, 4, 4], bf16, tag="w01")
            nc.vector.tensor_tensor(
                w01[:],
                te[:, :, 0, :].unsqueeze(3)
                    .broadcast_to((128, NBLK, 4, 4)),
                te[:, :, 1, :].unsqueeze(2)
                    .broadcast_to((128, NBLK, 4, 4)),
                op=AL.mult)
            w64 = sb.tile([128, NBLK, 16, 4], bf16, tag="w64")
            nc.vector.tensor_tensor(
                w64[:],
                w01[:].rearrange("p t a b -> p t (a b)").unsqueeze(3)
                    .broadcast_to((128, NBLK, 16, 4)),
                te[:, :, 2, :].unsqueeze(2)
                    .broadcast_to((128, NBLK, 16, 4)),
                op=AL.mult)

            # ---- cube contraction: bf16 multiply + fold + reduce ----
            mb = sb.tile([128, NBLK, 3, 64], bf16, tag="mb")
            nc.vector.tensor_tensor(
                mb[:],
                cubeb[:].rearrange("p t (c s) -> p t c s", s=64),
                w64[:].rearrange("p t a b -> p t (a b)").unsqueeze(2)
                    .broadcast_to((128, NBLK, 3, 64)),
                op=AL.mult)
            mbh = sb.tile([128, NBLK, 3, 32], bf16, tag="mbh")
            nc.vector.tensor_tensor(
                mbh[:], mb[:, :, :, 0:32],
                mb[:, :, :, 32:64], op=AL.add)
            col = sm.tile([128, NBLK, 3], fp32, tag="col")
            nc.vector.tensor_reduce(
                col[:], mbh[:].rearrange("p t c s -> p (t c) s"),
                axis=mybir.AxisListType.X, op=AL.add)

            # ---- hit mask + diff vs reference ----
            hm = sm.tile([128, NBLK], fp32, tag="hm")
            nc.vector.tensor_scalar(hm[:], nkmax[:], -HIT_THRESH,
                                    None, AL.is_gt)
            flat = sm.tile([128, NBLK, 3], fp32, tag="flat")
            nc.vector.tensor_tensor(
                flat[:], col[:],
                hm[:].unsqueeze(2).broadcast_to((128, NBLK, 3)),
                op=AL.mult)
            nc.vector.tensor_tensor(diff[:], flat[:], rs[:, :, :],
                                    op=AL.subtract)

            # ---- squared-error accumulate; host sums the 128 partitions ----
            sq = sb.tile([128, NBLK * 3], fp32, tag="sq")
            acc = sm.tile([128, 1], fp32, tag="acc")
            dv = diff[:].rearrange("p t c -> p (t c)")
            nc.vector.scalar_tensor_tensor(sq[:], dv, 1.0, dv, op0=AL.mult,
                                           op1=AL.mult, accum_out=acc[:])
            lsb = sm.tile([128, 1], fp32, tag="lsb")
            nc.gpsimd.partition_all_reduce(lsb[:], acc[:], channels=128,
                                           reduce_op=_bisa.ReduceOp.add)
            nc.sync.dma_start(out=lossp[:], in_=lsb[0:1, :])
            if probes:
                nc.sync.dma_start(out=p_nkmax[:], in_=st["nkmax"][:])
                nc.sync.dma_start(out=p_nk[:], in_=st["nk"][:])
                nc.sync.dma_start(out=p_cb[:],
                                  in_=cubeb[:].rearrange("p t c -> p (t c)"))
                nc.sync.dma_start(out=p_diff[:],
                                  in_=diff[:].rearrange("p t c -> p (t c)"))
                nc.sync.dma_start(out=p_acc[:], in_=acc[:])

        PIPE = True
        if loop_n is None or not PIPE:
            for _ in range(1 if loop_n is None else UNROLL):
                emit_tail(emit_front())
        else:
            prev = emit_front()
            for _ in range(UNROLL - 1):
                cur = emit_front()
                emit_tail(prev)
                prev = cur
            emit_tail(prev)

    nc.compile()
    return nc


def _binned_in_maps(np_inputs, geom, lists):
    """Host tables for the v4 binned program."""
    w0c, w1c, w2c, Dc, valid = (geom["w0c"], geom["w1c"], geom["w2c"],
                                geom["Dc"], geom["valid"])
    xs = ((np.arange(W, dtype=np.float64) + 0.5) / W * 2.0 - 1.0)
    ys = (1.0 - (np.arange(H, dtype=np.float64) + 0.5) / H * 2.0)
    dy = -1.0 / 64.0
    nbc = W // BC

    # per-face coefficient stacks [3(basis rows A,B,C), F+1] with poison row
    def ext(c):
        z = np.zeros((3, F + 1))
        z[:, :F] = c
        return z
    e0, e1, e2, ed = ext(w0c), ext(w1c), ext(w2c), ext(Dc)

    # tanh'd texture cube, c-major, bf16 [F+1, 192]
    cube = np.tanh(np.asarray(np_inputs["textures"][0], np.float64))
    cube = cube.reshape(F, TS, TS, TS, 3).transpose(0, 4, 1, 2, 3)
    cube_ext = np.zeros((F + 1, CUBE), np.float32)
    cube_ext[:F] = cube.reshape(F, CUBE)
    cube_bf = cube_ext.astype(ml_dtypes.bfloat16)

    pvalid = np.concatenate([valid, [False]])
    image_ref = np.asarray(np_inputs["image_ref"])

    # constant pixel basis, replicated into PE row groups 0/32/64/96
    j = np.arange(128) % 64
    r = (np.arange(128) // 64).astype(np.float64)
    xs0 = xs[j]
    pb3 = np.stack([np.ones(128), xs0, r]).astype(np.float32)  # [3,128]
    pb = np.zeros((12, 128), np.float32)
    for g in range(4):
        pb[3 * g:3 * g + 3] = pb3
    kk = np.broadcast_to(
        np.arange(4, dtype=np.float32), (128, 4)).copy()
    ident = np.eye(128, dtype=np.float32)
    identb = np.eye(128, dtype=ml_dtypes.bfloat16).view(np.uint16)
    identb = identb.reshape(128, 64, 2).astype(np.uint32)
    identb = (identb[:, :, 0] | (identb[:, :, 1] << 16)).view(np.float32)

    in_maps = []
    for c in range(NCORES):
        # sort blocks dense-first so window-1 real faces fit slots 0..39
        cnt = (lists[c] < F).sum(axis=1)
        perm = np.argsort(-cnt, kind="stable")
        assert np.sort(cnt)[::-1][WB:].max() <= 40
        li = lists[c][perm]                            # [NBLK, CAP]
        # per-block folded affine: A' = A + B*bj + C*py0 over basis (1,xs0,r)
        blkrow = perm // nbc
        bj = (perm % nbc).astype(np.float64)
        py0 = ys[c * TPC + blkrow * BR]
        rcb = np.zeros((3, NBLK, CAP, 4))
        cf = np.zeros((NBLK, CAP, CROW))
        for e_i, e in enumerate((e0, e1, e2)):
            A = e[0][li]; B = e[1][li]; C = e[2][li]   # [NBLK, CAP]
            Af = A + B * bj[:, None] + C * py0[:, None]
            rcb[0, :, :, e_i] = -KSCALE * Af
            rcb[1, :, :, e_i] = -KSCALE * B
            rcb[2, :, :, e_i] = -KSCALE * (C * dy)
            cf[:, :, 3 * e_i] = Af
            cf[:, :, 3 * e_i + 1] = B
            cf[:, :, 3 * e_i + 2] = C * dy
        A = ed[0][li]; B = ed[1][li]; C = ed[2][li]
        rcb[0, :, :, 3] = (A + B * bj[:, None] + C * py0[:, None]) + DSHIFT
        rcb[1, :, :, 3] = B
        rcb[2, :, :, 3] = C * dy
        # poison: padded slots and degenerate faces -> key 1e30
        poison = pvalid[li] == False                    # noqa: E712
        rcb[0][poison] = [1e30, 0.0, 0.0, 0.0]
        rcb[1][poison] = 0.0
        rcb[2][poison] = 0.0
        cf[poison] = 0.0
        rcb = rcb.reshape(3, NBLK * CAP * 4).astype(np.float32)
        rcbq = np.zeros((3, 128 + NBLK * CAP * 4), np.float32)
        rcbq[:, 0:128] = pb3
        rcbq[:, 128:] = rcb

        # block-diagonal slot table in section layout: even block t=2j on
        # partitions 0-63 at col j*SROW, odd t=2j+1 on partitions 64-127 at
        # ES + j*SROW; other halves zero so a full K=128 one-hot matmul
        # selects without cross-block bleed.
        # row = [cube bf16 | coef_hi bf16 | coef_lo bf16 (exact split)]
        SROW = CUBE + 24
        ES = (NBLK // 2) * SROW
        tabc = np.zeros((64, 2 * ES), ml_dtypes.bfloat16)
        cf32 = cf.astype(np.float32)
        cf_hi = cf32.astype(ml_dtypes.bfloat16)
        cf_lo = (cf32 - cf_hi.astype(np.float32)).astype(ml_dtypes.bfloat16)
        for t in range(NBLK):
            base = (t % 2) * ES + (t // 2) * SROW
            tabc[:, base:base + CUBE] = cube_bf[li[t]]
            tabc[:, base + CUBE:base + CUBE + 12] = cf_hi[t]
            tabc[:, base + CUBE + 12:base + SROW] = cf_lo[t]

        refsl = np.zeros((128, NBLK, 3), np.float32)
        for t in range(NBLK):
            br_, bj_ = divmod(int(perm[t]), nbc)
            rows = c * TPC + br_ * BR + np.arange(BR)
            cols = bj_ * BC + np.arange(BC)
            refsl[:, t, :] = image_ref[0][:, rows, :][:, :, cols] \
                .transpose(1, 2, 0).reshape(128, 3)

        cpkd = np.concatenate([
            pb3[1:2].T.astype(np.float32),       # xs0
            pb3[2:3].T.astype(np.float32),       # rvec
            kk,                                  # tent k table
            refsl.reshape(128, NBLK * 3),        # reference slice
            identb,                              # bf16 transpose identity
        ], axis=1)
        in_maps.append({"rcb": rcbq, "cpk": cpkd, "tabc": tabc})
    return in_maps


_last_exec_ns = None
_last_results = None
_last_in_maps = None


def kernel(vertices=None, textures=None, image_ref=None, faces=None,
           _trace=False, _probes=False, **kw):
    global _last_exec_ns, _last_results, _last_in_maps
    from concourse.bass_utils import run_bass_kernel_spmd

    vertices = np.asarray(vertices)
    textures = np.asarray(textures)
    image_ref = np.asarray(image_ref)
    faces = np.asarray(faces)
    np_inputs = {"vertices": vertices, "textures": textures,
                 "image_ref": image_ref, "faces": faces}

    geom = _geom(vertices, faces)
    lists = _bin_faces(geom)
    assert lists is not None, "bin overflow: CAP too small for this input"
    in_maps = _binned_in_maps(np_inputs, geom, lists)
    key = ("nc", _probes)
    if key not in _prog_cache:
        _prog_cache[key] = _build_binned(probes=_probes)
    nc = _prog_cache[key]
    _last_in_maps = in_maps
    res = run_bass_kernel_spmd(nc, in_maps, core_ids=list(range(NCORES)),
                               trace=_trace)
    _last_exec_ns = res.exec_time_ns
    _last_results = res
    total = np.float32(0.0)
    for r in res.results:
        total += np.float32(r["lossp"].reshape(()))
    return np.asarray(total, np.float32)
